# revision 1
# baseline (speedup 1.0000x reference)
"""Trainium2 Bass kernel for MllamaTextSelfAttention (B=1, S=2048, HID=4096,
32 Q heads / 8 KV heads, HD=128, RoPE, causal mask, GQA).

Sharding: tensor-parallel over heads across 8 NeuronCores. Core c computes
Q heads [4c, 4c+4) and KV head c, plus the matching slice of the output
projection; the 8 partial outputs are summed on the host.

Device dataflow (per core, all matmuls in fp32r):
  - qT/kT/vT = W^T-stationary projections -> [d, s] layouts
  - RoPE applied in [d, s] layout (partition-shifted rotate-half)
  - V transposed to natural [s, d] via PE transpose
  - S^T[k, q] = kT-tile.T @ qT-stripe  (k-partition layout)
  - E = exp(S^T + mask)  (no max subtraction; scores are O(10), mask -1e9
    underflows exp to exactly 0)
  - den[1, q] = ones.T @ E  (partition reduction on PE)
  - O^T[d, q] = V-tile.T @ E  accumulated over k tiles
  - O^T *= broadcast(1/den)  (rank-1 ones x recip matmul for the
    partition-direction broadcast)
  - Y[s, hid] = O^T-tiles.T @ woT  streamed to DRAM
"""

import math
import os
import sys

for _p in (
    "/opt/trn_rl_repo",
    "/root/.axon_site",
    "/root/.axon_site/_ro/trn_rl_repo",
    "/root/.axon_site/_ro/pypackages",
):
    if os.path.isdir(_p) and _p not in sys.path:
        sys.path.append(_p)

import numpy as np
from contextlib import ExitStack

import concourse.bass as bass
import concourse.tile as tile
from concourse import mybir
from concourse.bass_utils import run_bass_kernel_spmd
from concourse.masks import make_identity

F32 = mybir.dt.float32
FR = mybir.dt.float32r
ACTF = mybir.ActivationFunctionType

B, S, HID = 1, 2048, 4096
NH, NKV, HD = 32, 8, 128
NCORES = 8
QH = NH // NCORES          # 4 q heads per core
SS = 512                   # sequence stripe (matmul free dim)
NQS = S // SS              # 4 stripes
NKT = S // 128             # 16 k tiles
KH = HID // 128            # 32 hidden-dim k tiles
NEG = -1e9


def _split_multi_waits(nc: bass.Bass):
    """Walrus in this container encodes at most ONE sync-wait command per
    instruction. Hoist extra waits onto injected same-engine NoOps placed
    immediately before the instruction; engines are in-order so the
    semantics are unchanged."""
    n = 0
    for fn in nc.m.functions:
        for bb in fn.blocks:
            out = []
            for inst in bb.instructions:
                si = inst.sync_info
                if si is not None and si.on_wait and len(si.on_wait) > 1:
                    waits = list(si.on_wait)
                    for w in waits[:-1]:
                        n += 1
                        nop = mybir.InstNoOp(name=f"I-swait-{n}", ins=[], outs=[])
                        nop.engine = inst.engine
                        nop.sync_info = mybir.SyncInfo(on_wait=[w], on_update=[])
                        out.append(nop)
                    si.on_wait = [waits[-1]]
                out.append(inst)
            bb.instructions[:] = out
    return nc


_BUILD_CACHE = {}


def _build(causal: bool, split_waits: bool = True, loop_n=None) -> bass.Bass:
    key = (causal, split_waits, loop_n)
    if key in _BUILD_CACHE:
        return _BUILD_CACHE[key]

    nc = bass.Bass()
    hT = nc.dram_tensor("hT", [HID, S], F32, kind="ExternalInput")
    wqT = nc.dram_tensor("wqT", [HID, QH * HD], F32, kind="ExternalInput")
    wkT = nc.dram_tensor("wkT", [HID, HD], F32, kind="ExternalInput")
    wvT = nc.dram_tensor("wvT", [HID, HD], F32, kind="ExternalInput")
    woT = nc.dram_tensor("woT", [QH * HD, HID], F32, kind="ExternalInput")
    cosT = nc.dram_tensor("cosT", [HD, S], F32, kind="ExternalInput")
    sinT = nc.dram_tensor("sinT", [HD, S], F32, kind="ExternalInput")
    if causal:
        maskd = nc.dram_tensor("maskd", [128, 4 * SS], F32, kind="ExternalInput")
    else:
        maskT = nc.dram_tensor("maskT", [S, S], F32, kind="ExternalInput")
    y = nc.dram_tensor("y", [S, HID], F32, kind="ExternalOutput")

    with tile.TileContext(nc) as tc, ExitStack() as ctx:
        if loop_n is not None:
            # device-side repeat loop for dispatch-amortized timing
            ctx.enter_context(tc.For_i(0, loop_n, 1))
        # SWDGE (gpsimd) DMA inside a For_i fails this walrus' codegen, so
        # the timing variant issues everything from SP instead
        gp = nc.sync if loop_n is not None else nc.gpsimd
        outer = ctx.enter_context(tc.tile_pool(name="outer", bufs=1))
        qT = outer.tile([128, QH * S], FR)      # [d, h*s]
        kT = outer.tile([128, S], FR)           # [d, s]
        vT = outer.tile([128, S], F32)          # [d, s]
        v_sb = outer.tile([128, S], FR)         # [s-within-tile, t*d]
        ot = outer.tile([128, QH * S], FR)      # [d, h*s] normalized O^T

        # ---------------- phase 1: QKV projections ----------------
        with (
            tc.tile_pool(name="wqkv", bufs=1) as wp,
            tc.tile_pool(name="hstream", bufs=3) as hp,
            tc.tile_pool(name="ps1", bufs=1, space="PSUM") as pp1,
        ):
            wq_c = [wp.tile([128, 512], FR, name=f"wqc{k}", tag=f"wqc{k}") for k in range(KH)]
            wk_c = [wp.tile([128, HD], FR, name=f"wkc{k}", tag=f"wkc{k}") for k in range(KH)]
            wv_c = [wp.tile([128, HD], FR, name=f"wvc{k}", tag=f"wvc{k}") for k in range(KH)]
            for k in range(KH):
                gp.dma_start(wq_c[k][:], wqT[k * 128 : (k + 1) * 128, :].bitcast(FR))
                gp.dma_start(wk_c[k][:], wkT[k * 128 : (k + 1) * 128, :].bitcast(FR))
                gp.dma_start(wv_c[k][:], wvT[k * 128 : (k + 1) * 128, :].bitcast(FR))

            for n in range(NQS):
                psq = [
                    pp1.tile([128, SS], F32, name=f"psq{m}", tag=f"psq{m}")
                    for m in range(QH)
                ]
                psk = pp1.tile([128, SS], F32, tag="psk")
                psv = pp1.tile([128, SS], F32, tag="psv")
                for k in range(KH):
                    ht = hp.tile([128, SS], FR, tag="ht")
                    dma_eng = nc.sync if (k % 2 == 0) else gp
                    dma_eng.dma_start(
                        ht[:],
                        hT[k * 128 : (k + 1) * 128, n * SS : (n + 1) * SS].bitcast(FR),
                    )
                    st, sp = (k == 0), (k == KH - 1)
                    for m in range(QH):
                        nc.tensor.matmul(
                            psq[m][:],
                            wq_c[k][:, m * 128 : (m + 1) * 128],
                            ht[:],
                            start=st,
                            stop=sp,
                        )
                    nc.tensor.matmul(psk[:], wk_c[k][:], ht[:], start=st, stop=sp)
                    nc.tensor.matmul(psv[:], wv_c[k][:], ht[:], start=st, stop=sp)
                for m in range(QH):
                    nc.scalar.copy(
                        qT[:, m * S + n * SS : m * S + (n + 1) * SS], psq[m][:]
                    )
                nc.scalar.copy(kT[:, n * SS : (n + 1) * SS], psk[:])
                nc.scalar.copy(vT[:, n * SS : (n + 1) * SS], psv[:])

        # ---------------- phase 1.5: RoPE + V transpose ----------------
        with (
            tc.tile_pool(name="rope", bufs=1) as rp,
            tc.tile_pool(name="pst", bufs=2, space="PSUM") as ppt,
        ):
            cos_sb = rp.tile([128, S], F32)
            sin_sb = rp.tile([128, S], F32)
            nc.sync.dma_start(cos_sb[:], cosT[:, :])
            nc.sync.dma_start(sin_sb[:], sinT[:, :])
            for i in range(QH + 1):
                src = qT[:, i * S : (i + 1) * S] if i < QH else kT[:, :]
                rot = rp.tile([128, S], F32, tag="rot")
                tmp = rp.tile([128, S], F32, tag="tmp")
                # rotate_half in [d, s]: rot[0:64] = -src[64:128]; rot[64:128] = src[0:64]
                nc.vector.tensor_scalar_mul(rot[0:64, :], src[64:128, :], -1.0)
                nc.vector.tensor_copy(rot[64:128, :], src[0:64, :])
                nc.vector.tensor_mul(tmp[:], src, cos_sb[:])
                nc.vector.tensor_mul(rot[:], rot[:], sin_sb[:])
                nc.vector.tensor_add(src, tmp[:], rot[:])

            id_sb = rp.tile([128, 128], F32)
            make_identity(nc, id_sb[:])
            for t in range(NKT):
                pst = ppt.tile([128, 128], F32, tag="pst")
                nc.tensor.transpose(pst[:], vT[:, t * 128 : (t + 1) * 128], id_sb[:])
                nc.scalar.copy(v_sb[:, t * 128 : (t + 1) * 128], pst[:])

        # ---------------- phase 2: attention ----------------
        with (
            tc.tile_pool(name="att", bufs=1) as ap_,
            tc.tile_pool(name="epool", bufs=2 if causal else 1) as ep,
            tc.tile_pool(name="mrowp", bufs=1) as mp,
            tc.tile_pool(name="ps2s", bufs=4, space="PSUM") as pp2s,
            tc.tile_pool(name="ps2a", bufs=1, space="PSUM") as pp2a,
        ):
            ones_f32 = ap_.tile([128, 128], F32)
            nc.vector.memset(ones_f32[:], 1.0)
            ones_sb = ap_.tile([128, 128], FR)
            nc.vector.tensor_copy(ones_sb[:], ones_f32[:])
            if causal:
                md_sb = ap_.tile([128, 4 * SS], F32)
                nc.sync.dma_start(md_sb[:], maskd[:, :])

            for qs in range(NQS):
                nkt = 4 * qs + 4 if causal else NKT
                if not causal:
                    mrow = mp.tile([128, NKT * SS], F32, tag="mrow")
                    for t in range(NKT):
                        nc.sync.dma_start(
                            mrow[:, t * SS : (t + 1) * SS],
                            maskT[t * 128 : (t + 1) * 128, qs * SS : (qs + 1) * SS],
                        )
                for h in range(QH):
                    e = ep.tile([128, NKT * SS], FR, tag="e")
                    qsl = qT[:, h * S + qs * SS : h * S + (qs + 1) * SS]
                    for t in range(nkt):
                        pss = pp2s.tile([128, SS], F32, tag="pss")
                        nc.tensor.matmul(
                            pss[:],
                            kT[:, t * 128 : (t + 1) * 128],
                            qsl,
                            start=True,
                            stop=True,
                        )
                        dst = e[:, t * SS : (t + 1) * SS]
                        if causal and t >= 4 * qs:
                            j = t - 4 * qs
                            nc.vector.tensor_add(
                                dst, pss[:], md_sb[:, j * SS : (j + 1) * SS]
                            )
                            nc.scalar.activation(dst, dst, ACTF.Exp)
                        elif not causal:
                            nc.vector.tensor_add(
                                dst, pss[:], mrow[:, t * SS : (t + 1) * SS]
                            )
                            nc.scalar.activation(dst, dst, ACTF.Exp)
                        else:
                            nc.scalar.activation(dst, pss[:], ACTF.Exp)

                    psd = pp2a.tile([1, SS], F32, tag="psd")
                    pso = pp2a.tile([128, SS], F32, tag="pso")
                    for t in range(nkt):
                        er = e[:, t * SS : (t + 1) * SS]
                        st, sp = (t == 0), (t == nkt - 1)
                        nc.tensor.matmul(
                            psd[:], ones_sb[:, 0:1], er, start=st, stop=sp
                        )
                        nc.tensor.matmul(
                            pso[:],
                            v_sb[:, t * 128 : (t + 1) * 128],
                            er,
                            start=st,
                            stop=sp,
                        )
                    den = ap_.tile([1, SS], F32, tag="den")
                    nc.vector.tensor_copy(den[:], psd[:])
                    rec = ap_.tile([1, SS], FR, tag="rec")
                    with nc.allow_low_precision(reason="fp32r recip feeds matmul"):
                        nc.vector.reciprocal(rec[:], den[:])
                    psb = pp2a.tile([128, SS], F32, tag="psb")
                    nc.tensor.matmul(
                        psb[:],
                        ones_sb[0:1, 0:128],
                        rec[:],
                        start=True,
                        stop=True,
                    )
                    od = ot[:, h * S + qs * SS : h * S + (qs + 1) * SS]
                    nc.vector.tensor_copy(od, pso[:])
                    nc.vector.tensor_mul(od, od, psb[:])

        # ---------------- phase 3: output projection ----------------
        with (
            tc.tile_pool(name="wop", bufs=1) as wop,
            tc.tile_pool(name="yout", bufs=2) as yp,
            tc.tile_pool(name="ps3", bufs=2, space="PSUM") as pp3,
        ):
            wo_sb = wop.tile([128, QH * HID], FR)
            for hh in range(QH):
                gp.dma_start(
                    wo_sb[:, hh * HID : (hh + 1) * HID],
                    woT[hh * 128 : (hh + 1) * 128, :].bitcast(FR),
                )
            for st in range(NKT):
                yt = yp.tile([128, HID], F32, tag="yt")
                for nn in range(HID // SS):
                    psy = pp3.tile([128, SS], F32, tag="psy")
                    for hh in range(QH):
                        nc.tensor.matmul(
                            psy[:],
                            ot[:, hh * S + st * 128 : hh * S + (st + 1) * 128],
                            wo_sb[:, hh * HID + nn * SS : hh * HID + (nn + 1) * SS],
                            start=(hh == 0),
                            stop=(hh == QH - 1),
                        )
                    nc.scalar.copy(yt[:, nn * SS : (nn + 1) * SS], psy[:])
                eng = nc.sync if (st % 2 == 0) else gp
                eng.dma_start(y[st * 128 : (st + 1) * 128, :], yt[:])

    if split_waits:
        _split_multi_waits(nc)
    _BUILD_CACHE[key] = nc
    return nc


def _causal_mask_ref() -> np.ndarray:
    return np.triu(np.full((S, S), NEG, np.float32), k=1)


def _diag_mask_tiles() -> np.ndarray:
    p = np.arange(128, dtype=np.int64)[:, None]
    f = np.arange(SS, dtype=np.int64)[None, :]
    cols = [
        np.where(128 * j + p > f, np.float32(NEG), np.float32(0.0)) for j in range(4)
    ]
    return np.ascontiguousarray(np.concatenate(cols, axis=1).astype(np.float32))


def make_in_maps(hidden_states, attention_mask, cos, sin, wq, wk, wv, wo):
    """Host-side sharding/preprocessing. Returns (causal, in_maps)."""
    h = np.ascontiguousarray(np.asarray(hidden_states, dtype=np.float32)[0])
    m2 = np.ascontiguousarray(np.asarray(attention_mask, dtype=np.float32)[0, 0])
    wq = np.asarray(wq, dtype=np.float32)
    wk = np.asarray(wk, dtype=np.float32)
    wv = np.asarray(wv, dtype=np.float32)
    wo = np.asarray(wo, dtype=np.float32)

    causal = bool(np.array_equal(m2, _causal_mask_ref()))
    hT = np.ascontiguousarray(h.T)
    cosT = np.ascontiguousarray(np.asarray(cos, dtype=np.float32)[0].T)
    sinT = np.ascontiguousarray(np.asarray(sin, dtype=np.float32)[0].T)
    sc = np.float32(1.0 / math.sqrt(HD))
    if causal:
        md = _diag_mask_tiles()
    else:
        mT = np.ascontiguousarray(m2.T)

    in_maps = []
    for c in range(NCORES):
        im = {
            "hT": hT,
            "cosT": cosT,
            "sinT": sinT,
            "wqT": np.ascontiguousarray((wq[c * QH * HD : (c + 1) * QH * HD] * sc).T),
            "wkT": np.ascontiguousarray(wk[c * HD : (c + 1) * HD].T),
            "wvT": np.ascontiguousarray(wv[c * HD : (c + 1) * HD].T),
            "woT": np.ascontiguousarray(wo[:, c * QH * HD : (c + 1) * QH * HD].T),
        }
        if causal:
            im["maskd"] = md
        else:
            im["maskT"] = mT
        in_maps.append(im)
    return causal, in_maps


def kernel(hidden_states, attention_mask, cos, sin, wq, wk, wv, wo):
    causal, in_maps = make_in_maps(
        hidden_states, attention_mask, cos, sin, wq, wk, wv, wo
    )
    nc = _build(causal)
    res = run_bass_kernel_spmd(nc, in_maps, list(range(NCORES)))
    out = np.zeros((S, HID), np.float64)
    for c in range(NCORES):
        out += res.results[c]["y"].astype(np.float64)
    return out.reshape(B, S, HID).astype(np.float32)



# revision 6
# speedup vs baseline: 1.1551x; 1.1551x over previous
"""Trainium2 Bass kernel for MllamaTextSelfAttention (B=1, S=2048, HID=4096,
32 Q heads / 8 KV heads, HD=128, RoPE, causal mask, GQA).

Sharding: tensor-parallel over heads across 8 NeuronCores. Core c computes
Q heads [4c, 4c+4) and KV head c, plus the matching slice of the output
projection; the 8 partial outputs are summed on the host.

v2 dataflow (per core, bf16 matmul operands, fp32 PSUM accumulation):
  - All inputs host-packed into SBUF-tile layout and converted to bf16 so
    every weight/activation load is one large DMA (FWL-eligible stationaries).
  - Phase 1 (per 512-token stripe): QKV projections accumulate over the
    4096-dim contraction in 6 PSUM banks; RoPE is applied per-stripe right
    out of PSUM (ACT stage copy -> 4 DVE ops) so attention never waits on a
    separate RoPE pass. V is PE-transposed per stripe.
  - Phase 2 (per stripe, per head-pair): S^T[k,q] = kT.T @ qT with the kT
    tile stationary shared across the pair; exp on ACT straight from PSUM
    (bf16 out, no max subtraction -- causal zeroing via gpsimd affine_select
    after exp); denominator via ones-column matmul packed two heads into one
    PSUM bank (partitions 0 and 32); O^T = V-tile.T @ E accumulated over k;
    normalization via rank-1 ones x recip matmul + DVE multiply.
  - Phase 3: output projection with stationary-operand reuse: for each
    (s-tile, head) the ot slice stays stationary across 4 moving wo slices,
    2 groups of 4 PSUM banks double-buffered; copies to bf16 staging rotate
    over ACT/DVE/Pool; 16 row DMAs write the bf16 partial y.
"""

import math
import os
import sys

for _p in (
    "/opt/trn_rl_repo",
    "/root/.axon_site",
    "/root/.axon_site/_ro/trn_rl_repo",
    "/root/.axon_site/_ro/pypackages",
):
    if os.path.isdir(_p) and _p not in sys.path:
        sys.path.append(_p)

import numpy as np
import ml_dtypes
from contextlib import ExitStack

import concourse.bass as bass
import concourse.tile as tile
from concourse import mybir
from concourse.bass_utils import run_bass_kernel_spmd
from concourse.masks import make_identity

F32 = mybir.dt.float32
BF16 = mybir.dt.bfloat16
ACTF = mybir.ActivationFunctionType
ALU = mybir.AluOpType

B, S, HID = 1, 2048, 4096
NH, NKV, HD = 32, 8, 128
NCORES = 8
QH = NH // NCORES          # 4 q heads per core
SS = 512                   # sequence stripe
NQS = S // SS              # 4 stripes
NKT = S // 128             # 16 k tiles
KH = HID // 128            # 32 hidden-dim k tiles
NEG = -1e9


def _split_multi_waits(nc: bass.Bass):
    """Walrus in this container encodes at most ONE sync-wait command per
    instruction. Hoist extra waits onto injected same-engine NoOps placed
    immediately before the instruction; engines are in-order so the
    semantics are unchanged."""
    n = 0
    for fn in nc.m.functions:
        for bb in fn.blocks:
            out = []
            for inst in bb.instructions:
                si = inst.sync_info
                if si is not None and si.on_wait and len(si.on_wait) > 1:
                    waits = list(si.on_wait)
                    for w in waits[:-1]:
                        n += 1
                        nop = mybir.InstNoOp(name=f"I-swait-{n}", ins=[], outs=[])
                        nop.engine = inst.engine
                        nop.sync_info = mybir.SyncInfo(on_wait=[w], on_update=[])
                        out.append(nop)
                    si.on_wait = [waits[-1]]
                out.append(inst)
            bb.instructions[:] = out
    return nc


_BUILD_CACHE = {}


def _build(causal: bool, split_waits: bool = True, loop_n=None) -> bass.Bass:
    key = (causal, split_waits, loop_n)
    if key in _BUILD_CACHE:
        return _BUILD_CACHE[key]

    nc = bass.Bass()
    hS = [
        nc.dram_tensor(f"h{n}", [128, KH * SS], BF16, kind="ExternalInput")
        for n in range(NQS)
    ]
    wq = nc.dram_tensor("wq", [128, KH * SS], BF16, kind="ExternalInput")
    wk = nc.dram_tensor("wk", [128, KH * HD], BF16, kind="ExternalInput")
    wv = nc.dram_tensor("wv", [128, KH * HD], BF16, kind="ExternalInput")
    wo = nc.dram_tensor("wo", [128, QH * HID], BF16, kind="ExternalInput")
    trig = nc.dram_tensor("trig", [128, 2 * S], BF16, kind="ExternalInput")
    if not causal:
        maskT = nc.dram_tensor("maskT", [S, S], F32, kind="ExternalInput")
    y = nc.dram_tensor("y", [S, HID], BF16, kind="ExternalOutput")

    with tile.TileContext(nc) as tc, ExitStack() as ctx:
        if loop_n is not None:
            ctx.enter_context(tc.For_i(0, loop_n, 1))

        outer = ctx.enter_context(tc.tile_pool(name="outer", bufs=1))
        wq_sb = outer.tile([128, KH * SS], BF16)
        wk_sb = outer.tile([128, KH * HD], BF16)
        wv_sb = outer.tile([128, KH * HD], BF16)
        wo_sb = outer.tile([128, QH * HID], BF16)
        trig_sb = outer.tile([128, 2 * S], BF16)
        qT = outer.tile([128, QH * S], BF16)     # [d, h*s] rope'd
        kT = outer.tile([128, S], BF16)          # [d, s] rope'd
        v_sb = outer.tile([128, S], BF16)        # [s-within-tile, t*d]
        ot = outer.tile([128, QH * S], BF16)     # [d, h*s] normalized O^T
        ones_col = outer.tile([128, 1], BF16)
        ones_row = outer.tile([1, 128], BF16)
        id_bf = outer.tile([128, 128], BF16)

        # upfront bulk loads (SP-triggered, all single DMAs)
        nc.sync.dma_start(wq_sb[:], wq[:, :])
        nc.sync.dma_start(wk_sb[:], wk[:, :])
        nc.sync.dma_start(wv_sb[:], wv[:, :])
        nc.sync.dma_start(wo_sb[:], wo[:, :])
        nc.sync.dma_start(trig_sb[:], trig[:, :])
        cos_sb = trig_sb[:, 0:S]
        msin_sb = trig_sb[:, S : 2 * S]

        nc.vector.memset(ones_col[:], 1.0)
        nc.vector.memset(ones_row[:], 1.0)
        make_identity(nc, id_bf[:])

        # ---------------- phase 1: QKV projections + RoPE + V^T ----------
        with (
            tc.tile_pool(name="hstream", bufs=2) as hp,
            tc.tile_pool(name="stage", bufs=2) as sp_,
            tc.tile_pool(name="ps1", bufs=1, space="PSUM") as pp1,
            tc.tile_pool(name="pst", bufs=2, space="PSUM") as ppt,
        ):
            h_tiles = []
            for n in range(min(2, NQS)):
                ht = hp.tile([128, KH * SS], BF16, tag="ht")
                nc.sync.dma_start(ht[:], hS[n][:, :])
                h_tiles.append(ht)

            for n in range(NQS):
                ht = h_tiles[n]
                if n + 2 < NQS:
                    nxt = hp.tile([128, KH * SS], BF16, tag="ht")
                    nc.sync.dma_start(nxt[:], hS[n + 2][:, :])
                    h_tiles.append(nxt)

                psq = [
                    pp1.tile([128, SS], F32, name=f"psq{m}", tag=f"psq{m}")
                    for m in range(QH)
                ]
                psk = pp1.tile([128, SS], F32, tag="psk")
                psv = pp1.tile([128, SS], F32, tag="psv")
                for k in range(KH):
                    hsl = ht[:, k * SS : (k + 1) * SS]
                    st, sp = (k == 0), (k == KH - 1)
                    for m in range(QH):
                        nc.tensor.matmul(
                            psq[m][:],
                            wq_sb[:, k * SS + m * 128 : k * SS + (m + 1) * 128],
                            hsl,
                            start=st,
                            stop=sp,
                        )
                    nc.tensor.matmul(
                        psk[:], wk_sb[:, k * HD : (k + 1) * HD], hsl,
                        start=st, stop=sp,
                    )
                    nc.tensor.matmul(
                        psv[:], wv_sb[:, k * HD : (k + 1) * HD], hsl,
                        start=st, stop=sp,
                    )

                # RoPE per stripe: stage fp32 psum -> bf16, then 4 DVE ops
                cs = cos_sb[:, n * SS : (n + 1) * SS]
                ms = msin_sb[:, n * SS : (n + 1) * SS]
                for i in range(QH + 1):
                    src = psq[i][:] if i < QH else psk[:]
                    dst = (
                        qT[:, i * S + n * SS : i * S + (n + 1) * SS]
                        if i < QH
                        else kT[:, n * SS : (n + 1) * SS]
                    )
                    stg = sp_.tile([128, SS], BF16, tag="stg")
                    nc.scalar.copy(stg[:], src)
                    t1 = sp_.tile([128, SS], BF16, tag="t1")
                    t2 = sp_.tile([128, SS], BF16, tag="t2")
                    # rotate_half via partition-shifted copies (single-input
                    # ops allow mismatched base partitions), sign folded into
                    # the host-packed msin table
                    nc.vector.tensor_copy(t1[0:64, :], stg[64:128, :])
                    nc.vector.tensor_copy(t1[64:128, :], stg[0:64, :])
                    nc.vector.tensor_mul(t1[:], t1[:], ms)
                    nc.vector.tensor_mul(t2[:], stg[:], cs)
                    nc.vector.tensor_add(dst, t1[:], t2[:])

                # V transpose to [s, d] layout
                vb = sp_.tile([128, SS], BF16, tag="vb")
                nc.scalar.copy(vb[:], psv[:])
                for j in range(SS // 128):
                    t4 = 4 * n + j
                    pst = ppt.tile([128, 128], BF16, tag="pst")
                    nc.tensor.transpose(
                        pst[:], vb[:, j * 128 : (j + 1) * 128], id_bf[:]
                    )
                    nc.vector.tensor_copy(
                        v_sb[:, t4 * 128 : (t4 + 1) * 128], pst[:]
                    )

        # ---------------- phase 2: attention ----------------
        with (
            tc.tile_pool(name="epool", bufs=2) as ep,
            tc.tile_pool(name="att", bufs=2) as ap_,
            tc.tile_pool(name="mrowp", bufs=1) as mp,
            tc.tile_pool(name="ps2s", bufs=2, space="PSUM") as pp2s,
            tc.tile_pool(name="ps2a", bufs=1, space="PSUM") as pp2a,
        ):
            for qs in range(NQS):
                nkt = 4 * qs + 4 if causal else NKT
                if not causal:
                    mrow = mp.tile([128, NKT * SS], F32, tag="mrow")
                    for t in range(NKT):
                        nc.sync.dma_start(
                            mrow[:, t * SS : (t + 1) * SS],
                            maskT[t * 128 : (t + 1) * 128, qs * SS : (qs + 1) * SS],
                        )
                for hp_ in range(QH // 2):
                    h0, h1 = 2 * hp_, 2 * hp_ + 1
                    e0 = ep.tile([128, NKT * SS], BF16, tag="e0")
                    e1 = ep.tile([128, NKT * SS], BF16, tag="e1")
                    q0 = qT[:, h0 * S + qs * SS : h0 * S + (qs + 1) * SS]
                    q1 = qT[:, h1 * S + qs * SS : h1 * S + (qs + 1) * SS]
                    for t in range(nkt):
                        ksl = kT[:, t * 128 : (t + 1) * 128]
                        for (hh, qsl, ee) in ((0, q0, e0), (1, q1, e1)):
                            pss = pp2s.tile([128, SS], F32, tag=f"pss{hh}")
                            nc.tensor.matmul(pss[:], ksl, qsl, start=True, stop=True)
                            dst = ee[:, t * SS : (t + 1) * SS]
                            if causal:
                                nc.scalar.activation(dst, pss[:], ACTF.Exp)
                                if t >= 4 * qs:
                                    dd = t - 4 * qs
                                    # keep where q - k >= 0: iota = j - p - 128*dd
                                    nc.gpsimd.affine_select(
                                        out=dst,
                                        in_=dst,
                                        pattern=[[1, SS]],
                                        compare_op=ALU.is_ge,
                                        fill=0.0,
                                        base=-(128 * dd),
                                        channel_multiplier=-1,
                                    )
                            else:
                                nc.vector.tensor_add(
                                    pss[:], pss[:], mrow[:, t * SS : (t + 1) * SS]
                                )
                                nc.scalar.activation(dst, pss[:], ACTF.Exp)

                    psd = pp2a.tile([64, SS], F32, tag="psd")
                    pso0 = pp2a.tile([128, SS], F32, tag="pso0")
                    pso1 = pp2a.tile([128, SS], F32, tag="pso1")
                    for t in range(nkt):
                        st, sp = (t == 0), (t == nkt - 1)
                        vsl = v_sb[:, t * 128 : (t + 1) * 128]
                        e0t = e0[:, t * SS : (t + 1) * SS]
                        e1t = e1[:, t * SS : (t + 1) * SS]
                        nc.tensor.matmul(
                            psd[0:1, :], ones_col[:], e0t, start=st, stop=sp,
                            skip_group_check=True,
                        )
                        nc.tensor.matmul(
                            psd[32:33, :], ones_col[:], e1t, start=st, stop=sp,
                            skip_group_check=True,
                        )
                        nc.tensor.matmul(pso0[:], vsl, e0t, start=st, stop=sp)
                        nc.tensor.matmul(pso1[:], vsl, e1t, start=st, stop=sp)

                    with nc.allow_low_precision(reason="bf16 recip feeds matmul"):
                        rec0 = ap_.tile([1, SS], BF16, tag="rec0")
                        rec1 = ap_.tile([1, SS], BF16, tag="rec1")
                        nc.vector.reciprocal(rec0[:], psd[0:1, :])
                        nc.vector.reciprocal(rec1[:], psd[32:33, :])
                    psb0 = pp2s.tile([128, SS], F32, tag="pss0")
                    psb1 = pp2s.tile([128, SS], F32, tag="pss1")
                    nc.tensor.matmul(
                        psb0[:], ones_row[:], rec0[:], start=True, stop=True
                    )
                    nc.tensor.matmul(
                        psb1[:], ones_row[:], rec1[:], start=True, stop=True
                    )
                    for (hh, pso, psb) in ((h0, pso0, psb0), (h1, pso1, psb1)):
                        od = ot[:, hh * S + qs * SS : hh * S + (qs + 1) * SS]
                        nc.vector.tensor_copy(od, pso[:])
                        nc.vector.tensor_mul(od, od, psb[:])

        # ---------------- phase 3: output projection ----------------
        with (
            tc.tile_pool(name="yout", bufs=2) as yp,
            tc.tile_pool(name="ps3", bufs=1, space="PSUM") as pp3,
        ):
            cp_engines = (
                lambda o, i: nc.scalar.copy(o, i),
                lambda o, i: nc.vector.tensor_copy(o, i),
            )
            rr = 0
            for st in range(NKT):
                yt = yp.tile([128, HID], BF16, tag="yt")
                for g in range(2):
                    psy = [
                        pp3.tile([128, SS], F32, name=f"psy{g}{j}", tag=f"psy{g}{j}")
                        for j in range(4)
                    ]
                    for hh in range(QH):
                        osl = ot[:, hh * S + st * 128 : hh * S + (st + 1) * 128]
                        for j in range(4):
                            nn = 4 * g + j
                            nc.tensor.matmul(
                                psy[j][:],
                                osl,
                                wo_sb[:, hh * HID + nn * SS : hh * HID + (nn + 1) * SS],
                                start=(hh == 0),
                                stop=(hh == QH - 1),
                            )
                    for j in range(4):
                        nn = 4 * g + j
                        cp_engines[rr % 2](
                            yt[:, nn * SS : (nn + 1) * SS], psy[j][:]
                        )
                        rr += 1
                nc.sync.dma_start(y[st * 128 : (st + 1) * 128, :], yt[:])

    if split_waits:
        _split_multi_waits(nc)
    _BUILD_CACHE[key] = nc
    return nc


def _causal_mask_ref() -> np.ndarray:
    return np.triu(np.full((S, S), NEG, np.float32), k=1)


def _pack(a: np.ndarray) -> np.ndarray:
    """[R, W] with R = 128*r -> [128, r*W] SBUF tile layout, bf16."""
    r = a.shape[0] // 128
    w = a.shape[1]
    out = a.reshape(r, 128, w).transpose(1, 0, 2).reshape(128, r * w)
    return np.ascontiguousarray(out.astype(ml_dtypes.bfloat16))


def make_in_maps(hidden_states, attention_mask, cos, sin, wq, wk, wv, wo):
    """Host-side sharding/packing. Returns (causal, in_maps)."""
    h = np.asarray(hidden_states, dtype=np.float32)[0]
    m2 = np.ascontiguousarray(np.asarray(attention_mask, dtype=np.float32)[0, 0])
    wq = np.asarray(wq, dtype=np.float32)
    wk = np.asarray(wk, dtype=np.float32)
    wv = np.asarray(wv, dtype=np.float32)
    wo = np.asarray(wo, dtype=np.float32)

    causal = bool(np.array_equal(m2, _causal_mask_ref()))
    hT = h.T  # [HID, S]
    cosT = np.asarray(cos, dtype=np.float32)[0].T  # [HD, S]
    sinT = np.asarray(sin, dtype=np.float32)[0].T
    msinT = np.concatenate([-sinT[0:64], sinT[64:128]], axis=0)
    trig = np.ascontiguousarray(
        np.concatenate([cosT, msinT], axis=1).astype(ml_dtypes.bfloat16)
    )
    sc = np.float32(1.0 / math.sqrt(HD))

    h_stripes = [
        _pack(np.ascontiguousarray(hT[:, n * SS : (n + 1) * SS])) for n in range(NQS)
    ]
    if not causal:
        mT = np.ascontiguousarray(m2.T)

    in_maps = []
    for c in range(NCORES):
        im = {
            "trig": trig,
            "wq": _pack(np.ascontiguousarray((wq[c * QH * HD : (c + 1) * QH * HD] * sc).T)),
            "wk": _pack(np.ascontiguousarray(wk[c * HD : (c + 1) * HD].T)),
            "wv": _pack(np.ascontiguousarray(wv[c * HD : (c + 1) * HD].T)),
            "wo": _pack(np.ascontiguousarray(wo[:, c * QH * HD : (c + 1) * QH * HD].T)),
        }
        for n in range(NQS):
            im[f"h{n}"] = h_stripes[n]
        if not causal:
            im["maskT"] = mT
        in_maps.append(im)
    return causal, in_maps


def kernel(hidden_states, attention_mask, cos, sin, wq, wk, wv, wo):
    causal, in_maps = make_in_maps(
        hidden_states, attention_mask, cos, sin, wq, wk, wv, wo
    )
    nc = _build(causal)
    res = run_bass_kernel_spmd(nc, in_maps, list(range(NCORES)))
    out = np.zeros((S, HID), np.float64)
    for c in range(NCORES):
        out += res.results[c]["y"].astype(np.float64)
    return out.reshape(B, S, HID).astype(np.float32)


# revision 12
# speedup vs baseline: 1.2701x; 1.0995x over previous
"""Trainium2 Bass kernel for MllamaTextSelfAttention (B=1, S=2048, HID=4096,
32 Q heads / 8 KV heads, HD=128, RoPE, causal mask, GQA).

Sharding: tensor-parallel over heads across 8 NeuronCores. Core c computes
Q heads [4c, 4c+4) and KV head c, plus the matching slice of the output
projection; the 8 partial outputs are summed on the host.

v3 dataflow (per core, bf16 matmul operands, fp32 PSUM accumulation):
  - All inputs host-packed into SBUF-tile layout and converted to bf16 so
    every load is one large DMA (FWL-eligible stationaries), ordered so the
    first matmul's dependencies land first; wo loads during attention.
  - Phase 1 processes stripes in PAIRS with weight-stationary sharing: for
    each hidden k-tile the wk/wv/wq slice is loaded once and multiplied
    against both stripes' h tiles (the redundant second InstLdweights is
    removed by a post-scheduling dedupe pass). KV pass (4 PSUM banks) then
    Q pass (2 tags x 2 bufs); RoPE applied per (pair, tensor) right out of
    PSUM; V PE-transposed after the Q pass reusing the q PSUM banks.
  - Phase 2 (per stripe, per head-pair): S^T[k,q] = kT.T @ qT with the kT
    tile stationary shared across the pair; diagonal tiles compute only the
    live [128*dd, 512) column sub-range; exp on ACT straight from PSUM
    (bf16 out), causal zeroing via a precomputed 0/1 bf16 mask multiply on
    DVE; denominator via ones-column matmul packing two heads into one PSUM
    bank (partitions 0/32); O^T = V-tile.T @ E accumulated over k;
    normalization via rank-1 ones x recip matmul + DVE multiply.
  - Phase 3: output projection with stationary reuse: per (s-tile, head)
    the ot slice stays stationary across 4 moving wo slices, 2 groups of 4
    PSUM banks double-buffered; PSUM->bf16 copies alternate ACT/DVE; 16 row
    DMAs write the bf16 partial y.
"""

import math
import os
import sys

for _p in (
    "/opt/trn_rl_repo",
    "/root/.axon_site",
    "/root/.axon_site/_ro/trn_rl_repo",
    "/root/.axon_site/_ro/pypackages",
):
    if os.path.isdir(_p) and _p not in sys.path:
        sys.path.append(_p)

import numpy as np
import ml_dtypes
from contextlib import ExitStack

import concourse.bass as bass
import concourse.tile as tile
from concourse import mybir
from concourse.bass_utils import run_bass_kernel_spmd
from concourse.masks import make_identity

F32 = mybir.dt.float32
BF16 = mybir.dt.bfloat16
ACTF = mybir.ActivationFunctionType
ALU = mybir.AluOpType

B, S, HID = 1, 2048, 4096
NH, NKV, HD = 32, 8, 128
NCORES = 8
QH = NH // NCORES          # 4 q heads per core
SS = 512                   # sequence stripe
NQS = S // SS              # 4 stripes
NKT = S // 128             # 16 k tiles
KH = HID // 128            # 32 hidden-dim k tiles
HH_ = KH * SS // 2         # half-stripe free size (16 k-tiles)
NEG = -1e9


def _split_multi_waits(nc: bass.Bass):
    """Walrus in this container encodes at most ONE sync-wait command per
    instruction. Hoist extra waits onto injected same-engine NoOps placed
    immediately before the instruction; engines are in-order so the
    semantics are unchanged."""
    n = 0
    for fn in nc.m.functions:
        for bb in fn.blocks:
            out = []
            for inst in bb.instructions:
                si = inst.sync_info
                if si is not None and si.on_wait and len(si.on_wait) > 1:
                    waits = list(si.on_wait)
                    for w in waits[:-1]:
                        n += 1
                        nop = mybir.InstNoOp(name=f"I-swait-{n}", ins=[], outs=[])
                        nop.engine = inst.engine
                        nop.sync_info = mybir.SyncInfo(on_wait=[w], on_update=[])
                        out.append(nop)
                    si.on_wait = [waits[-1]]
                out.append(inst)
            bb.instructions[:] = out
    return nc


def _dedupe_ldweights(nc: bass.Bass):
    """The Tile legalizer emits one InstLdweights per matmul. Consecutive
    matmuls issued with the same stationary operand reload the PE array
    needlessly (~54-107ns each on HW). Drop the redundant loads: the PE
    array retains its weights across InstMatmult. Redundant loads carrying
    sync info become PE NoOps (sync position in the PE stream preserved);
    sync-free ones are deleted outright."""
    n = 0
    for fn in nc.m.functions:
        for bb in fn.blocks:
            out = []
            last_sig = None
            for inst in bb.instructions:
                if getattr(inst, "engine", None) == mybir.EngineType.PE:
                    nm = type(inst).__name__
                    if nm == "InstLdweights":
                        w = inst.ins[-1]
                        sig = (
                            str(w.memref),
                            w.offset,
                            str(w.ap),
                            str(w.dtype),
                            str(inst.perf_mode),
                            str(inst.is_transpose),
                        )
                        if sig == last_sig:
                            si = inst.sync_info
                            if si is not None and (si.on_wait or si.on_update):
                                n += 1
                                nop = mybir.InstNoOp(
                                    name=f"I-dlw-{n}", ins=[], outs=[]
                                )
                                nop.engine = mybir.EngineType.PE
                                nop.sync_info = si
                                out.append(nop)
                            continue
                        last_sig = sig
                out.append(inst)
            bb.instructions[:] = out
    return nc


_BUILD_CACHE = {}
DEDUPE = True


def _build(causal: bool, split_waits: bool = True, loop_n=None) -> bass.Bass:
    key = (causal, split_waits, loop_n, DEDUPE)
    if key in _BUILD_CACHE:
        return _BUILD_CACHE[key]

    nc = bass.Bass()
    hS = [
        nc.dram_tensor(f"h{n}", [128, KH * SS], BF16, kind="ExternalInput")
        for n in range(NQS)
    ]
    wq = nc.dram_tensor("wq", [128, KH * SS], BF16, kind="ExternalInput")
    wk = nc.dram_tensor("wk", [128, KH * HD], BF16, kind="ExternalInput")
    wv = nc.dram_tensor("wv", [128, KH * HD], BF16, kind="ExternalInput")
    wo = nc.dram_tensor("wo", [128, QH * HID], BF16, kind="ExternalInput")
    trig = nc.dram_tensor("trig", [128, 2 * S], BF16, kind="ExternalInput")
    if not causal:
        maskT = nc.dram_tensor("maskT", [S, S], F32, kind="ExternalInput")
    y = nc.dram_tensor("y", [S, HID], BF16, kind="ExternalOutput")

    with tile.TileContext(nc) as tc, ExitStack() as ctx:
        if loop_n is not None:
            ctx.enter_context(tc.For_i(0, loop_n, 1))

        outer = ctx.enter_context(tc.tile_pool(name="outer", bufs=1))
        wq_sb = outer.tile([128, KH * SS], BF16)
        wk_sb = outer.tile([128, KH * HD], BF16)
        wv_sb = outer.tile([128, KH * HD], BF16)
        trig_sb = outer.tile([128, 2 * S], BF16)
        qT = outer.tile([128, QH * S], BF16)     # [d, h*s] rope'd
        kT = outer.tile([128, S], BF16)          # [d, s] rope'd
        v_sb = outer.tile([128, S], BF16)        # [s-within-tile, t*d]
        ot = outer.tile([128, QH * S], BF16)     # [d, h*s] normalized O^T
        ones_col = outer.tile([128, 1], BF16)
        ones_row = outer.tile([1, 128], BF16)
        id_bf = outer.tile([128, 128], BF16)
        d01 = outer.tile([128, QH * SS], BF16)   # causal 0/1 mask per dd

        # upfront bulk loads in consumption order (SP-triggered)
        nc.sync.dma_start(wk_sb[:], wk[:, :])
        nc.sync.dma_start(wv_sb[:], wv[:, :])

        nc.vector.memset(ones_col[:], 1.0)
        nc.vector.memset(ones_row[:], 1.0)
        make_identity(nc, id_bf[:])
        nc.gpsimd.memset(d01[:], 1.0)
        for dd in range(QH):
            # keep 1.0 where q - k >= 0: iota = j - p - 128*dd
            nc.gpsimd.affine_select(
                out=d01[:, dd * SS : (dd + 1) * SS],
                in_=d01[:, dd * SS : (dd + 1) * SS],
                pattern=[[1, SS]],
                compare_op=ALU.is_ge,
                fill=0.0,
                base=-(128 * dd),
                channel_multiplier=-1,
            )

        # ------- phase 1: QKV projections (stripe pairs) + RoPE + V^T -----
        with (
            tc.tile_pool(name="hstream", bufs=6) as hp,
            tc.tile_pool(name="stage", bufs=2) as sp_,
            tc.tile_pool(name="ps1", bufs=1, space="PSUM") as pp1,
            tc.tile_pool(name="psq", bufs=2, space="PSUM") as ppq,
        ):
            def rope(src_psum, dst):
                stg = sp_.tile([128, SS], BF16, tag="stg")
                nc.scalar.copy(stg[:], src_psum)
                t1 = sp_.tile([128, SS], BF16, tag="t1")
                t2 = sp_.tile([128, SS], BF16, tag="t2")
                nc.vector.tensor_copy(t1[0:64, :], stg[64:128, :])
                nc.vector.tensor_copy(t1[64:128, :], stg[0:64, :])
                nc.vector.tensor_mul(t1[:], t1[:], rope.ms)
                nc.vector.tensor_mul(t2[:], stg[:], rope.cs)
                nc.vector.tensor_add(dst, t1[:], t2[:])

            # h half-stripe tiles, DMA'd in KV-pass consumption order
            half = {}
            def load_half(n, j):
                t_ = hp.tile([128, HH_], BF16, name=f"hh{n}{j}", tag="hh")
                nc.sync.dma_start(t_[:], hS[n][:, j * HH_ : (j + 1) * HH_])
                half[(n, j)] = t_

            load_half(0, 0)
            load_half(1, 0)
            load_half(0, 1)
            load_half(1, 1)
            nc.sync.dma_start(trig_sb[:], trig[:, :])
            nc.sync.dma_start(wq_sb[:], wq[:, :])

            def hsl(n, k):
                t_ = half[(n, k // 16)]
                o = (k % 16) * SS
                return t_[:, o : o + SS]

            for pr in range(NQS // 2):
                a, b = 2 * pr, 2 * pr + 1
                # ---- KV pass: wk/wv stationary shared across the pair ----
                psk = [pp1.tile([128, SS], F32, name=f"psk{i}", tag=f"psk{i}") for i in range(2)]
                psv = [pp1.tile([128, SS], F32, name=f"psv{i}", tag=f"psv{i}") for i in range(2)]
                for k in range(KH):
                    st_, sp2 = (k == 0), (k == KH - 1)
                    wksl = wk_sb[:, k * HD : (k + 1) * HD]
                    wvsl = wv_sb[:, k * HD : (k + 1) * HD]
                    nc.tensor.matmul(psk[0][:], wksl, hsl(a, k), start=st_, stop=sp2)
                    nc.tensor.matmul(psk[1][:], wksl, hsl(b, k), start=st_, stop=sp2)
                    nc.tensor.matmul(psv[0][:], wvsl, hsl(a, k), start=st_, stop=sp2)
                    nc.tensor.matmul(psv[1][:], wvsl, hsl(b, k), start=st_, stop=sp2)

                vbf = []
                for i, n in enumerate((a, b)):
                    rope.cs = trig_sb[:, n * SS : (n + 1) * SS]
                    rope.ms = trig_sb[:, S + n * SS : S + (n + 1) * SS]
                    rope(psk[i][:], kT[:, n * SS : (n + 1) * SS])
                    vb = sp_.tile([128, SS], BF16, name=f"vb{i}", tag=f"vb{i}")
                    nc.scalar.copy(vb[:], psv[i][:])
                    vbf.append(vb)

                # ---- Q pass: wq stationary shared across the pair ----
                for m in range(QH):
                    psqa = ppq.tile([128, SS], F32, tag="qa")
                    psqb = ppq.tile([128, SS], F32, tag="qb")
                    for k in range(KH):
                        st_, sp2 = (k == 0), (k == KH - 1)
                        wqsl = wq_sb[:, k * SS + m * 128 : k * SS + (m + 1) * 128]
                        nc.tensor.matmul(psqa[:], wqsl, hsl(a, k), start=st_, stop=sp2)
                        nc.tensor.matmul(psqb[:], wqsl, hsl(b, k), start=st_, stop=sp2)
                    for n, ps in ((a, psqa), (b, psqb)):
                        rope.cs = trig_sb[:, n * SS : (n + 1) * SS]
                        rope.ms = trig_sb[:, S + n * SS : S + (n + 1) * SS]
                        rope(ps[:], qT[:, m * S + n * SS : m * S + (n + 1) * SS])
                    if pr == 0 and m < 2:
                        # prefetch next pair's first halves during Q pass
                        load_half(2 + m, 0)
                if pr == 0:
                    load_half(2, 1)
                    load_half(3, 1)

                # ---- V transposes (reuse q psum banks) ----
                for i in range(2):
                    n = (a, b)[i]
                    for j in range(SS // 128):
                        t4 = 4 * n + j
                        pst = ppq.tile([128, 128], BF16, tag=("qa", "qb")[i])
                        nc.tensor.transpose(
                            pst[:], vbf[i][:, j * 128 : (j + 1) * 128], id_bf[:]
                        )
                        nc.vector.tensor_copy(
                            v_sb[:, t4 * 128 : (t4 + 1) * 128], pst[:]
                        )

        # ---------------- phase 2: attention ----------------
        with tc.tile_pool(name="wop", bufs=1) as wop:
          wo_sb = wop.tile([128, QH * HID], BF16)
          nc.sync.dma_start(wo_sb[:], wo[:, :])
          with (
            tc.tile_pool(name="epool", bufs=2) as ep,
            tc.tile_pool(name="att", bufs=2) as ap_,
            tc.tile_pool(name="mrowp", bufs=1) as mp,
            tc.tile_pool(name="ps2s", bufs=2, space="PSUM") as pp2s,
            tc.tile_pool(name="ps2a", bufs=1, space="PSUM") as pp2a,
          ):
            for qs in range(NQS):
                nkt = 4 * qs + 4 if causal else NKT
                if not causal:
                    mrow = mp.tile([128, NKT * SS], F32, tag="mrow")
                    for t in range(NKT):
                        nc.sync.dma_start(
                            mrow[:, t * SS : (t + 1) * SS],
                            maskT[t * 128 : (t + 1) * 128, qs * SS : (qs + 1) * SS],
                        )
                for hp_ in range(QH // 2):
                    h0, h1 = 2 * hp_, 2 * hp_ + 1
                    e0 = ep.tile([128, NKT * SS], BF16, tag="e0")
                    e1 = ep.tile([128, NKT * SS], BF16, tag="e1")
                    for t in range(nkt):
                        ksl = kT[:, t * 128 : (t + 1) * 128]
                        diag = causal and t >= 4 * qs
                        off = 128 * (t - 4 * qs) if diag else 0
                        w_ = SS - off
                        for hh, hid_ in ((0, h0), (1, h1)):
                            qsl = qT[:, hid_ * S + qs * SS + off : hid_ * S + (qs + 1) * SS]
                            pss = pp2s.tile([128, SS], F32, tag=f"pss{hh}")
                            ee = (e0, e1)[hh]
                            nc.tensor.matmul(
                                pss[:, 0:w_], ksl, qsl, start=True, stop=True
                            )
                            dst = ee[:, t * SS + off : (t + 1) * SS]
                            if causal:
                                nc.scalar.activation(dst, pss[:, 0:w_], ACTF.Exp)
                                if diag:
                                    dd = t - 4 * qs
                                    if off:
                                        nc.vector.memset(
                                            ee[:, t * SS : t * SS + off], 0.0
                                        )
                                    nc.vector.tensor_mul(
                                        dst, dst,
                                        d01[:, dd * SS + off : (dd + 1) * SS],
                                    )
                            else:
                                nc.vector.tensor_add(
                                    pss[:], pss[:], mrow[:, t * SS : (t + 1) * SS]
                                )
                                nc.scalar.activation(dst, pss[:], ACTF.Exp)

                    psd = pp2a.tile([64, SS], F32, tag="psd")
                    pso0 = pp2a.tile([128, SS], F32, tag="pso0")
                    pso1 = pp2a.tile([128, SS], F32, tag="pso1")
                    for t in range(nkt):
                        st_, sp2 = (t == 0), (t == nkt - 1)
                        vsl = v_sb[:, t * 128 : (t + 1) * 128]
                        e0t = e0[:, t * SS : (t + 1) * SS]
                        e1t = e1[:, t * SS : (t + 1) * SS]
                        nc.tensor.matmul(
                            psd[0:1, :], ones_col[:], e0t, start=st_, stop=sp2,
                            skip_group_check=True,
                        )
                        nc.tensor.matmul(
                            psd[32:33, :], ones_col[:], e1t, start=st_, stop=sp2,
                            skip_group_check=True,
                        )
                        nc.tensor.matmul(pso0[:], vsl, e0t, start=st_, stop=sp2)
                        nc.tensor.matmul(pso1[:], vsl, e1t, start=st_, stop=sp2)

                    with nc.allow_low_precision(reason="bf16 recip feeds matmul"):
                        rec0 = ap_.tile([1, SS], BF16, tag="rec0")
                        rec1 = ap_.tile([1, SS], BF16, tag="rec1")
                        nc.vector.reciprocal(rec0[:], psd[0:1, :])
                        nc.vector.reciprocal(rec1[:], psd[32:33, :])
                    psb0 = pp2s.tile([128, SS], F32, tag="pss0")
                    psb1 = pp2s.tile([128, SS], F32, tag="pss1")
                    nc.tensor.matmul(
                        psb0[:], ones_row[:], rec0[:], start=True, stop=True
                    )
                    nc.tensor.matmul(
                        psb1[:], ones_row[:], rec1[:], start=True, stop=True
                    )
                    for (hh, pso, psb) in ((h0, pso0, psb0), (h1, pso1, psb1)):
                        od = ot[:, hh * S + qs * SS : hh * S + (qs + 1) * SS]
                        nc.vector.tensor_copy(od, pso[:])
                        nc.vector.tensor_mul(od, od, psb[:])

          # ------------- phase 3: output projection -------------
          with (
                tc.tile_pool(name="yout", bufs=2) as yp,
                tc.tile_pool(name="ps3", bufs=1, space="PSUM") as pp3,
          ):
                cp_engines = (
                    lambda o, i: nc.scalar.copy(o, i),
                    lambda o, i: nc.vector.tensor_copy(o, i),
                )
                rr = 0
                for st in range(NKT):
                    yt = yp.tile([128, HID], BF16, tag="yt")
                    for g in range(2):
                        psy = [
                            pp3.tile([128, SS], F32, name=f"psy{g}{j}", tag=f"psy{g}{j}")
                            for j in range(4)
                        ]
                        for hh in range(QH):
                            osl = ot[:, hh * S + st * 128 : hh * S + (st + 1) * 128]
                            for j in range(4):
                                nn = 4 * g + j
                                nc.tensor.matmul(
                                    psy[j][:],
                                    osl,
                                    wo_sb[:, hh * HID + nn * SS : hh * HID + (nn + 1) * SS],
                                    start=(hh == 0),
                                    stop=(hh == QH - 1),
                                )
                        for j in range(4):
                            nn = 4 * g + j
                            cp_engines[rr % 2](
                                yt[:, nn * SS : (nn + 1) * SS], psy[j][:]
                            )
                            rr += 1
                    nc.sync.dma_start(y[st * 128 : (st + 1) * 128, :], yt[:])

    if DEDUPE:
        _dedupe_ldweights(nc)
    if split_waits:
        _split_multi_waits(nc)
    _BUILD_CACHE[key] = nc
    return nc


def _causal_mask_ref() -> np.ndarray:
    return np.triu(np.full((S, S), NEG, np.float32), k=1)


def _pack(a: np.ndarray) -> np.ndarray:
    """[R, W] with R = 128*r -> [128, r*W] SBUF tile layout, bf16."""
    r = a.shape[0] // 128
    w = a.shape[1]
    out = a.reshape(r, 128, w).transpose(1, 0, 2).reshape(128, r * w)
    return np.ascontiguousarray(out.astype(ml_dtypes.bfloat16))


def make_in_maps(hidden_states, attention_mask, cos, sin, wq, wk, wv, wo):
    """Host-side sharding/packing. Returns (causal, in_maps)."""
    h = np.asarray(hidden_states, dtype=np.float32)[0]
    m2 = np.ascontiguousarray(np.asarray(attention_mask, dtype=np.float32)[0, 0])
    wq = np.asarray(wq, dtype=np.float32)
    wk = np.asarray(wk, dtype=np.float32)
    wv = np.asarray(wv, dtype=np.float32)
    wo = np.asarray(wo, dtype=np.float32)

    causal = bool(np.array_equal(m2, _causal_mask_ref()))
    hT = h.T  # [HID, S]
    cosT = np.asarray(cos, dtype=np.float32)[0].T  # [HD, S]
    sinT = np.asarray(sin, dtype=np.float32)[0].T
    msinT = np.concatenate([-sinT[0:64], sinT[64:128]], axis=0)
    trig = np.ascontiguousarray(
        np.concatenate([cosT, msinT], axis=1).astype(ml_dtypes.bfloat16)
    )
    sc = np.float32(1.0 / math.sqrt(HD))

    h_stripes = [
        _pack(np.ascontiguousarray(hT[:, n * SS : (n + 1) * SS])) for n in range(NQS)
    ]
    if not causal:
        mT = np.ascontiguousarray(m2.T)

    in_maps = []
    for c in range(NCORES):
        im = {
            "trig": trig,
            "wq": _pack(np.ascontiguousarray((wq[c * QH * HD : (c + 1) * QH * HD] * sc).T)),
            "wk": _pack(np.ascontiguousarray(wk[c * HD : (c + 1) * HD].T)),
            "wv": _pack(np.ascontiguousarray(wv[c * HD : (c + 1) * HD].T)),
            "wo": _pack(np.ascontiguousarray(wo[:, c * QH * HD : (c + 1) * QH * HD].T)),
        }
        for n in range(NQS):
            im[f"h{n}"] = h_stripes[n]
        if not causal:
            im["maskT"] = mT
        in_maps.append(im)
    return causal, in_maps


def kernel(hidden_states, attention_mask, cos, sin, wq, wk, wv, wo):
    causal, in_maps = make_in_maps(
        hidden_states, attention_mask, cos, sin, wq, wk, wv, wo
    )
    nc = _build(causal)
    res = run_bass_kernel_spmd(nc, in_maps, list(range(NCORES)))
    out = np.zeros((S, HID), np.float64)
    for c in range(NCORES):
        out += res.results[c]["y"].astype(np.float64)
    return out.reshape(B, S, HID).astype(np.float32)


# revision 13
# speedup vs baseline: 1.3870x; 1.0920x over previous
"""Trainium2 Bass kernel for MllamaTextSelfAttention (B=1, S=2048, HID=4096,
32 Q heads / 8 KV heads, HD=128, RoPE, causal mask, GQA).

Sharding: tensor-parallel over heads across 8 NeuronCores. Core c computes
Q heads [4c, 4c+4) and KV head c, plus the matching slice of the output
projection; the 8 partial outputs are summed on the host.

v3 dataflow (per core, bf16 matmul operands, fp32 PSUM accumulation):
  - All inputs host-packed into SBUF-tile layout and converted to bf16 so
    every load is one large DMA (FWL-eligible stationaries), ordered so the
    first matmul's dependencies land first; wo loads during attention.
  - Phase 1 processes stripes in PAIRS with weight-stationary sharing: for
    each hidden k-tile the wk/wv/wq slice is loaded once and multiplied
    against both stripes' h tiles (the redundant second InstLdweights is
    removed by a post-scheduling dedupe pass). KV pass (4 PSUM banks) then
    Q pass (2 tags x 2 bufs); RoPE applied per (pair, tensor) right out of
    PSUM; V PE-transposed after the Q pass reusing the q PSUM banks.
  - Phase 2 (per stripe, per head-pair): S^T[k,q] = kT.T @ qT with the kT
    tile stationary shared across the pair; diagonal tiles compute only the
    live [128*dd, 512) column sub-range; exp on ACT straight from PSUM
    (bf16 out), causal zeroing via a precomputed 0/1 bf16 mask multiply on
    DVE; denominator via ones-column matmul packing two heads into one PSUM
    bank (partitions 0/32); O^T = V-tile.T @ E accumulated over k;
    normalization via rank-1 ones x recip matmul + DVE multiply.
  - Phase 3: output projection with stationary reuse: per (s-tile, head)
    the ot slice stays stationary across 4 moving wo slices, 2 groups of 4
    PSUM banks double-buffered; PSUM->bf16 copies alternate ACT/DVE; 16 row
    DMAs write the bf16 partial y.
"""

import math
import os
import sys

for _p in (
    "/opt/trn_rl_repo",
    "/root/.axon_site",
    "/root/.axon_site/_ro/trn_rl_repo",
    "/root/.axon_site/_ro/pypackages",
):
    if os.path.isdir(_p) and _p not in sys.path:
        sys.path.append(_p)

import numpy as np
import ml_dtypes
from contextlib import ExitStack

import concourse.bass as bass
import concourse.tile as tile
from concourse import mybir
from concourse.bass_utils import run_bass_kernel_spmd
from concourse.masks import make_identity

F32 = mybir.dt.float32
BF16 = mybir.dt.bfloat16
ACTF = mybir.ActivationFunctionType
ALU = mybir.AluOpType

B, S, HID = 1, 2048, 4096
NH, NKV, HD = 32, 8, 128
NCORES = 8
QH = NH // NCORES          # 4 q heads per core
SS = 512                   # sequence stripe
NQS = S // SS              # 4 stripes
NKT = S // 128             # 16 k tiles
KH = HID // 128            # 32 hidden-dim k tiles
HH_ = KH * SS // 2         # half-stripe free size (16 k-tiles)
NEG = -1e9


def _split_multi_waits(nc: bass.Bass):
    """Walrus in this container encodes at most ONE sync-wait command per
    instruction. Hoist extra waits onto injected same-engine NoOps placed
    immediately before the instruction; engines are in-order so the
    semantics are unchanged."""
    n = 0
    for fn in nc.m.functions:
        for bb in fn.blocks:
            out = []
            for inst in bb.instructions:
                si = inst.sync_info
                if si is not None and si.on_wait and len(si.on_wait) > 1:
                    waits = list(si.on_wait)
                    for w in waits[:-1]:
                        n += 1
                        nop = mybir.InstNoOp(name=f"I-swait-{n}", ins=[], outs=[])
                        nop.engine = inst.engine
                        nop.sync_info = mybir.SyncInfo(on_wait=[w], on_update=[])
                        out.append(nop)
                    si.on_wait = [waits[-1]]
                out.append(inst)
            bb.instructions[:] = out
    return nc


def _dedupe_ldweights(nc: bass.Bass):
    """The Tile legalizer emits one InstLdweights per matmul. Consecutive
    matmuls issued with the same stationary operand reload the PE array
    needlessly (~54-107ns each on HW). Drop the redundant loads: the PE
    array retains its weights across InstMatmult. Redundant loads carrying
    sync info become PE NoOps (sync position in the PE stream preserved);
    sync-free ones are deleted outright."""
    n = 0
    for fn in nc.m.functions:
        for bb in fn.blocks:
            out = []
            last_sig = None
            for inst in bb.instructions:
                if getattr(inst, "engine", None) == mybir.EngineType.PE:
                    nm = type(inst).__name__
                    if nm == "InstLdweights":
                        w = inst.ins[-1]
                        sig = (
                            str(w.memref),
                            w.offset,
                            str(w.ap),
                            str(w.dtype),
                            str(inst.perf_mode),
                            str(inst.is_transpose),
                            str(getattr(inst, "tile_position", None)),
                            str(getattr(inst, "tile_size", None)),
                        )
                        # a load into a sub-array tile leaves other tile
                        # positions' weights unknown to this tracker —
                        # only full-array loads are safe dedupe anchors
                        if sig[6] != "(0, 0)" or sig[7] != "(128, 128)":
                            last_sig = None
                            out.append(inst)
                            continue
                        if sig == last_sig:
                            si = inst.sync_info
                            if si is not None and (si.on_wait or si.on_update):
                                n += 1
                                nop = mybir.InstNoOp(
                                    name=f"I-dlw-{n}", ins=[], outs=[]
                                )
                                nop.engine = mybir.EngineType.PE
                                nop.sync_info = si
                                out.append(nop)
                            continue
                        last_sig = sig
                out.append(inst)
            bb.instructions[:] = out
    return nc


_BUILD_CACHE = {}
DEDUPE = True


def _build(causal: bool, split_waits: bool = True, loop_n=None) -> bass.Bass:
    key = (causal, split_waits, loop_n, DEDUPE)
    if key in _BUILD_CACHE:
        return _BUILD_CACHE[key]

    nc = bass.Bass()
    hS = [
        nc.dram_tensor(f"h{n}", [128, KH * SS], BF16, kind="ExternalInput")
        for n in range(NQS)
    ]
    wq = nc.dram_tensor("wq", [128, KH * SS], BF16, kind="ExternalInput")
    wk = nc.dram_tensor("wk", [128, KH * HD], BF16, kind="ExternalInput")
    wv = nc.dram_tensor("wv", [128, KH * HD], BF16, kind="ExternalInput")
    wo = nc.dram_tensor("wo", [128, QH * HID], BF16, kind="ExternalInput")
    trig = nc.dram_tensor("trig", [128, 2 * S], BF16, kind="ExternalInput")
    if not causal:
        maskT = nc.dram_tensor("maskT", [S, S], F32, kind="ExternalInput")
    y = nc.dram_tensor("y", [S, HID], BF16, kind="ExternalOutput")

    with tile.TileContext(nc) as tc, ExitStack() as ctx:
        if loop_n is not None:
            ctx.enter_context(tc.For_i(0, loop_n, 1))

        outer = ctx.enter_context(tc.tile_pool(name="outer", bufs=1))
        wq_sb = outer.tile([128, KH * SS], BF16)
        wk_sb = outer.tile([128, KH * HD], BF16)
        wv_sb = outer.tile([128, KH * HD], BF16)
        trig_sb = outer.tile([128, 2 * S], BF16)
        qT = outer.tile([128, QH * S], BF16)     # [d, h*s] rope'd
        kT = outer.tile([128, S], BF16)          # [d, s] rope'd
        v_sb = outer.tile([128, S], BF16)        # [s-within-tile, t*d]
        ot = outer.tile([128, QH * S], BF16)     # [d, h*s] normalized O^T
        ones_col = outer.tile([128, 1], BF16)
        ones_row = outer.tile([1, 128], BF16)
        id_bf = outer.tile([128, 128], BF16)
        d01 = outer.tile([128, QH * SS], BF16)   # causal 0/1 mask per dd

        # upfront bulk loads in consumption order (SP-triggered)
        nc.sync.dma_start(wk_sb[:], wk[:, :])
        nc.sync.dma_start(wv_sb[:], wv[:, :])

        nc.vector.memset(ones_col[:], 1.0)
        nc.vector.memset(ones_row[:], 1.0)
        make_identity(nc, id_bf[:])
        nc.gpsimd.memset(d01[:], 1.0)
        for dd in range(QH):
            # keep 1.0 where q - k >= 0: iota = j - p - 128*dd
            nc.gpsimd.affine_select(
                out=d01[:, dd * SS : (dd + 1) * SS],
                in_=d01[:, dd * SS : (dd + 1) * SS],
                pattern=[[1, SS]],
                compare_op=ALU.is_ge,
                fill=0.0,
                base=-(128 * dd),
                channel_multiplier=-1,
            )

        # ------- phase 1: QKV projections (stripe pairs) + RoPE + V^T -----
        with (
            tc.tile_pool(name="hstream", bufs=6) as hp,
            tc.tile_pool(name="stage", bufs=2) as sp_,
            tc.tile_pool(name="ps1", bufs=1, space="PSUM") as pp1,
            tc.tile_pool(name="psq", bufs=2, space="PSUM") as ppq,
        ):
            def rope(src_psum, dst):
                stg = sp_.tile([128, SS], BF16, tag="stg")
                nc.scalar.copy(stg[:], src_psum)
                t1 = sp_.tile([128, SS], BF16, tag="t1")
                t2 = sp_.tile([128, SS], BF16, tag="t2")
                nc.vector.tensor_copy(t1[0:64, :], stg[64:128, :])
                nc.vector.tensor_copy(t1[64:128, :], stg[0:64, :])
                nc.vector.tensor_mul(t1[:], t1[:], rope.ms)
                nc.vector.tensor_mul(t2[:], stg[:], rope.cs)
                nc.vector.tensor_add(dst, t1[:], t2[:])

            # h half-stripe tiles, DMA'd in KV-pass consumption order
            half = {}
            def load_half(n, j):
                t_ = hp.tile([128, HH_], BF16, name=f"hh{n}{j}", tag="hh")
                nc.sync.dma_start(t_[:], hS[n][:, j * HH_ : (j + 1) * HH_])
                half[(n, j)] = t_

            load_half(0, 0)
            load_half(1, 0)
            load_half(0, 1)
            load_half(1, 1)
            nc.sync.dma_start(trig_sb[:], trig[:, :])
            nc.sync.dma_start(wq_sb[:], wq[:, :])

            def hsl(n, k):
                t_ = half[(n, k // 16)]
                o = (k % 16) * SS
                return t_[:, o : o + SS]

            for pr in range(NQS // 2):
                a, b = 2 * pr, 2 * pr + 1
                # ---- KV pass: wk/wv stationary shared across the pair ----
                psk = [pp1.tile([128, SS], F32, name=f"psk{i}", tag=f"psk{i}") for i in range(2)]
                psv = [pp1.tile([128, SS], F32, name=f"psv{i}", tag=f"psv{i}") for i in range(2)]
                for k in range(KH):
                    st_, sp2 = (k == 0), (k == KH - 1)
                    wksl = wk_sb[:, k * HD : (k + 1) * HD]
                    wvsl = wv_sb[:, k * HD : (k + 1) * HD]
                    nc.tensor.matmul(psk[0][:], wksl, hsl(a, k), start=st_, stop=sp2)
                    nc.tensor.matmul(psk[1][:], wksl, hsl(b, k), start=st_, stop=sp2)
                    nc.tensor.matmul(psv[0][:], wvsl, hsl(a, k), start=st_, stop=sp2)
                    nc.tensor.matmul(psv[1][:], wvsl, hsl(b, k), start=st_, stop=sp2)

                vbf = []
                for i, n in enumerate((a, b)):
                    rope.cs = trig_sb[:, n * SS : (n + 1) * SS]
                    rope.ms = trig_sb[:, S + n * SS : S + (n + 1) * SS]
                    rope(psk[i][:], kT[:, n * SS : (n + 1) * SS])
                    vb = sp_.tile([128, SS], BF16, name=f"vb{i}", tag=f"vb{i}")
                    nc.scalar.copy(vb[:], psv[i][:])
                    vbf.append(vb)

                # ---- Q pass: wq stationary shared across the pair ----
                for m in range(QH):
                    psqa = ppq.tile([128, SS], F32, tag="qa")
                    psqb = ppq.tile([128, SS], F32, tag="qb")
                    for k in range(KH):
                        st_, sp2 = (k == 0), (k == KH - 1)
                        wqsl = wq_sb[:, k * SS + m * 128 : k * SS + (m + 1) * 128]
                        nc.tensor.matmul(psqa[:], wqsl, hsl(a, k), start=st_, stop=sp2)
                        nc.tensor.matmul(psqb[:], wqsl, hsl(b, k), start=st_, stop=sp2)
                    for n, ps in ((a, psqa), (b, psqb)):
                        rope.cs = trig_sb[:, n * SS : (n + 1) * SS]
                        rope.ms = trig_sb[:, S + n * SS : S + (n + 1) * SS]
                        rope(ps[:], qT[:, m * S + n * SS : m * S + (n + 1) * SS])
                    if pr == 0 and m < 2:
                        # prefetch next pair's first halves during Q pass
                        load_half(2 + m, 0)
                if pr == 0:
                    load_half(2, 1)
                    load_half(3, 1)

                # ---- V transposes (reuse q psum banks) ----
                for i in range(2):
                    n = (a, b)[i]
                    for j in range(SS // 128):
                        t4 = 4 * n + j
                        pst = ppq.tile([128, 128], BF16, tag=("qa", "qb")[i])
                        nc.tensor.transpose(
                            pst[:], vbf[i][:, j * 128 : (j + 1) * 128], id_bf[:]
                        )
                        nc.vector.tensor_copy(
                            v_sb[:, t4 * 128 : (t4 + 1) * 128], pst[:]
                        )

        # ---------------- phase 2: attention ----------------
        with tc.tile_pool(name="wop", bufs=1) as wop:
          wo_sb = wop.tile([128, QH * HID], BF16)
          nc.sync.dma_start(wo_sb[:], wo[:, :])
          with (
            tc.tile_pool(name="epool", bufs=2) as ep,
            tc.tile_pool(name="att", bufs=2) as ap_,
            tc.tile_pool(name="mrowp", bufs=1) as mp,
            tc.tile_pool(name="ps2s", bufs=2, space="PSUM") as pp2s,
            tc.tile_pool(name="ps2a", bufs=1, space="PSUM") as pp2a,
          ):
            for qs in range(NQS):
                nkt = 4 * qs + 4 if causal else NKT
                if not causal:
                    mrow = mp.tile([128, NKT * SS], F32, tag="mrow")
                    for t in range(NKT):
                        nc.sync.dma_start(
                            mrow[:, t * SS : (t + 1) * SS],
                            maskT[t * 128 : (t + 1) * 128, qs * SS : (qs + 1) * SS],
                        )
                for hp_ in range(QH // 2):
                    h0, h1 = 2 * hp_, 2 * hp_ + 1
                    e0 = ep.tile([128, NKT * SS], BF16, tag="e0")
                    e1 = ep.tile([128, NKT * SS], BF16, tag="e1")
                    for t in range(nkt):
                        ksl = kT[:, t * 128 : (t + 1) * 128]
                        diag = causal and t >= 4 * qs
                        off = 128 * (t - 4 * qs) if diag else 0
                        w_ = SS - off
                        for hh, hid_ in ((0, h0), (1, h1)):
                            qsl = qT[:, hid_ * S + qs * SS + off : hid_ * S + (qs + 1) * SS]
                            pss = pp2s.tile([128, SS], F32, tag=f"pss{hh}")
                            ee = (e0, e1)[hh]
                            nc.tensor.matmul(
                                pss[:, 0:w_], ksl, qsl, start=True, stop=True
                            )
                            dst = ee[:, t * SS + off : (t + 1) * SS]
                            if causal:
                                nc.scalar.activation(dst, pss[:, 0:w_], ACTF.Exp)
                                if diag:
                                    dd = t - 4 * qs
                                    if off:
                                        nc.vector.memset(
                                            ee[:, t * SS : t * SS + off], 0.0
                                        )
                                    nc.vector.tensor_mul(
                                        dst, dst,
                                        d01[:, dd * SS + off : (dd + 1) * SS],
                                    )
                            else:
                                nc.vector.tensor_add(
                                    pss[:], pss[:], mrow[:, t * SS : (t + 1) * SS]
                                )
                                nc.scalar.activation(dst, pss[:], ACTF.Exp)

                    psd = pp2a.tile([64, SS], F32, tag="psd")
                    pso0 = pp2a.tile([128, SS], F32, tag="pso0")
                    pso1 = pp2a.tile([128, SS], F32, tag="pso1")
                    for t in range(nkt):
                        st_, sp2 = (t == 0), (t == nkt - 1)
                        vsl = v_sb[:, t * 128 : (t + 1) * 128]
                        e0t = e0[:, t * SS : (t + 1) * SS]
                        e1t = e1[:, t * SS : (t + 1) * SS]
                        nc.tensor.matmul(
                            psd[0:1, :], ones_col[:], e0t, start=st_, stop=sp2,
                            skip_group_check=True,
                        )
                        nc.tensor.matmul(
                            psd[32:33, :], ones_col[:], e1t, start=st_, stop=sp2,
                            skip_group_check=True,
                        )
                        nc.tensor.matmul(pso0[:], vsl, e0t, start=st_, stop=sp2)
                        nc.tensor.matmul(pso1[:], vsl, e1t, start=st_, stop=sp2)

                    with nc.allow_low_precision(reason="bf16 recip feeds matmul"):
                        rec0 = ap_.tile([1, SS], BF16, tag="rec0")
                        rec1 = ap_.tile([1, SS], BF16, tag="rec1")
                        nc.vector.reciprocal(rec0[:], psd[0:1, :])
                        nc.vector.reciprocal(rec1[:], psd[32:33, :])
                    psb0 = pp2s.tile([128, SS], F32, tag="pss0")
                    psb1 = pp2s.tile([128, SS], F32, tag="pss1")
                    nc.tensor.matmul(
                        psb0[:], ones_row[:], rec0[:], start=True, stop=True
                    )
                    nc.tensor.matmul(
                        psb1[:], ones_row[:], rec1[:], start=True, stop=True
                    )
                    for (hh, pso, psb) in ((h0, pso0, psb0), (h1, pso1, psb1)):
                        od = ot[:, hh * S + qs * SS : hh * S + (qs + 1) * SS]
                        nc.vector.tensor_copy(od, pso[:])
                        nc.vector.tensor_mul(od, od, psb[:])

          # ------------- phase 3: output projection -------------
          with (
                tc.tile_pool(name="yout", bufs=2) as yp,
                tc.tile_pool(name="ps3", bufs=1, space="PSUM") as pp3,
          ):
                cp_engines = (
                    lambda o, i: nc.scalar.copy(o, i),
                    lambda o, i: nc.vector.tensor_copy(o, i),
                )
                rr = 0
                for st in range(NKT):
                    yt = yp.tile([128, HID], BF16, tag="yt")
                    for g in range(2):
                        psy = [
                            pp3.tile([128, SS], F32, name=f"psy{g}{j}", tag=f"psy{g}{j}")
                            for j in range(4)
                        ]
                        for hh in range(QH):
                            osl = ot[:, hh * S + st * 128 : hh * S + (st + 1) * 128]
                            for j in range(4):
                                nn = 4 * g + j
                                nc.tensor.matmul(
                                    psy[j][:],
                                    osl,
                                    wo_sb[:, hh * HID + nn * SS : hh * HID + (nn + 1) * SS],
                                    start=(hh == 0),
                                    stop=(hh == QH - 1),
                                )
                        for j in range(4):
                            nn = 4 * g + j
                            cp_engines[rr % 2](
                                yt[:, nn * SS : (nn + 1) * SS], psy[j][:]
                            )
                            rr += 1
                    nc.sync.dma_start(y[st * 128 : (st + 1) * 128, :], yt[:])

    if DEDUPE:
        _dedupe_ldweights(nc)
    if split_waits:
        _split_multi_waits(nc)
    _BUILD_CACHE[key] = nc
    return nc


def _causal_mask_ref() -> np.ndarray:
    return np.triu(np.full((S, S), NEG, np.float32), k=1)


def _pack(a: np.ndarray) -> np.ndarray:
    """[R, W] with R = 128*r -> [128, r*W] SBUF tile layout, bf16."""
    r = a.shape[0] // 128
    w = a.shape[1]
    out = a.reshape(r, 128, w).transpose(1, 0, 2).reshape(128, r * w)
    return np.ascontiguousarray(out.astype(ml_dtypes.bfloat16))


def make_in_maps(hidden_states, attention_mask, cos, sin, wq, wk, wv, wo):
    """Host-side sharding/packing. Returns (causal, in_maps)."""
    h = np.asarray(hidden_states, dtype=np.float32)[0]
    m2 = np.ascontiguousarray(np.asarray(attention_mask, dtype=np.float32)[0, 0])
    wq = np.asarray(wq, dtype=np.float32)
    wk = np.asarray(wk, dtype=np.float32)
    wv = np.asarray(wv, dtype=np.float32)
    wo = np.asarray(wo, dtype=np.float32)

    causal = bool(np.array_equal(m2, _causal_mask_ref()))
    hT = h.T  # [HID, S]
    cosT = np.asarray(cos, dtype=np.float32)[0].T  # [HD, S]
    sinT = np.asarray(sin, dtype=np.float32)[0].T
    msinT = np.concatenate([-sinT[0:64], sinT[64:128]], axis=0)
    trig = np.ascontiguousarray(
        np.concatenate([cosT, msinT], axis=1).astype(ml_dtypes.bfloat16)
    )
    sc = np.float32(1.0 / math.sqrt(HD))

    h_stripes = [
        _pack(np.ascontiguousarray(hT[:, n * SS : (n + 1) * SS])) for n in range(NQS)
    ]
    if not causal:
        mT = np.ascontiguousarray(m2.T)

    in_maps = []
    for c in range(NCORES):
        im = {
            "trig": trig,
            "wq": _pack(np.ascontiguousarray((wq[c * QH * HD : (c + 1) * QH * HD] * sc).T)),
            "wk": _pack(np.ascontiguousarray(wk[c * HD : (c + 1) * HD].T)),
            "wv": _pack(np.ascontiguousarray(wv[c * HD : (c + 1) * HD].T)),
            "wo": _pack(np.ascontiguousarray(wo[:, c * QH * HD : (c + 1) * QH * HD].T)),
        }
        for n in range(NQS):
            im[f"h{n}"] = h_stripes[n]
        if not causal:
            im["maskT"] = mT
        in_maps.append(im)
    return causal, in_maps


def kernel(hidden_states, attention_mask, cos, sin, wq, wk, wv, wo):
    causal, in_maps = make_in_maps(
        hidden_states, attention_mask, cos, sin, wq, wk, wv, wo
    )
    nc = _build(causal)
    res = run_bass_kernel_spmd(nc, in_maps, list(range(NCORES)))
    out = np.zeros((S, HID), np.float64)
    for c in range(NCORES):
        out += res.results[c]["y"].astype(np.float64)
    return out.reshape(B, S, HID).astype(np.float32)
